# revision 6
# baseline (speedup 1.0000x reference)
"""nn_GatedGCNNet Trainium2 Bass kernel.

B=8, N=10000, E=160000, C=128. Data-parallel over batch: one batch element
per NeuronCore (8 cores), graph structure replicated.

Math (per batch element b, all linear ops folded to exploit linearity of the
scatter-sum):
    x        = X @ w1
    aggr     = icnt * ((sum_{e: tgt=n} ew_e * X[src_e]) @ (w1 @ v)) * w2
    out      = X @ (w1 @ u) + aggr
    BN over (batch, channel) per node  -> cross-core AllReduce of [m1; m2]
    result   = x + relu((out - mean) * rsqrt(var + eps))

Device pipeline per core:
  - gather raw X rows (bf16) straight from HBM with dma_gather (edge order
    sorted by target, CPU-precomputed int16 indices)
  - scatter-sum via TensorE matmuls: lhsT = gathered edge-tile [128e, 128c],
    rhs = narrow segment matrix [128e, 16] whose values are ew*icnt
    (CPU-precomputed), accumulated in PSUM per 512-target chunk
  - out = XbT.T @ (w1@u) + GT.T @ (w1@v*w2) per 128-node tile
  - per-node stats via free-axis DVE reduces, 80KB AllReduce, ACT-fused
    normalize+relu, DVE residual add, cast-to-fp32 DMA out.
"""
import sys

if "/opt/trn_rl_repo" not in sys.path:
    sys.path.append("/opt/trn_rl_repo")

import hashlib
import numpy as np
import ml_dtypes

bf16 = ml_dtypes.bfloat16

B, N, E, C = 8, 10000, 160000, 128
EPS = 1e-5
NTILE = (N + 127) // 128            # 79 node tiles
NP = NTILE * 128                    # 10112 padded nodes
CHUNK_T = 512                       # targets per PSUM chunk
NCHUNK = (N + CHUNK_T - 1) // CHUNK_T   # 20
W = 16                              # segment-matrix window width
NCORES = 8
DENOM = float(B * C)                # BN reduces over batch*channel = 1024

_state = None


def _preprocess(edge_index, edge_weight):
    src = np.asarray(edge_index[0]).astype(np.int64)
    tgt = np.asarray(edge_index[1]).astype(np.int64)
    ew = np.asarray(edge_weight, dtype=np.float32)
    counts = np.bincount(tgt, minlength=N)
    icnt = (1.0 / np.maximum(counts, 1)).astype(np.float32)
    order = np.argsort(tgt, kind="stable")
    srcs = src[order].astype(np.int16)
    tgts = tgt[order]
    wvals = (ew[order] * icnt[tgts]).astype(np.float32)

    bounds = np.searchsorted(tgts, np.arange(0, CHUNK_T * (NCHUNK + 1), CHUNK_T))
    tile_w0 = []
    tile_chunk = []
    e_tile = np.empty(E, np.int64)
    e_row = np.empty(E, np.int64)
    chunk_tiles = []
    for q in range(NCHUNK):
        lo, hi = int(bounds[q]), int(bounds[q + 1])
        t_lo = q * CHUNK_T
        chw = min(CHUNK_T, N - t_lo)
        ts = len(tile_w0)
        loc = (tgts[lo:hi] - t_lo).astype(np.int64)
        e = lo
        while e < hi:
            w0 = min(int(loc[e - lo]), chw - W)
            stop = lo + int(np.searchsorted(loc, w0 + W, side="left"))
            te_ = min(e + 128, stop, hi)
            tid = len(tile_w0)
            tile_w0.append(w0)
            tile_chunk.append(q)
            e_tile[e:te_] = tid
            e_row[e:te_] = np.arange(te_ - e)
            e = te_
        chunk_tiles.append((ts, len(tile_w0)))
    T = len(tile_w0)

    idx_tiles = np.zeros((T, 128), np.int16)
    idx_tiles[e_tile, e_row] = srcs
    w0arr = np.asarray(tile_w0, np.int64)
    qarr = np.asarray(tile_chunk, np.int64)
    locw = tgts - qarr[e_tile] * CHUNK_T - w0arr[e_tile]
    seg = np.zeros((128, T * W), np.float32)
    seg[e_row, e_tile * W + locw] = wvals
    seg16 = np.ascontiguousarray(seg.astype(bf16))

    idxw = np.zeros((16, T * 8), np.int16)
    for ts, te in chunk_tiles:
        blk = idx_tiles[ts:te].reshape(-1)
        idxw[:, ts * 8: te * 8] = blk.reshape(-1, 16).T
    idxw = np.ascontiguousarray(idxw)

    return dict(chunk_tiles=chunk_tiles, tile_w0=tile_w0, T=T, idxw=idxw, seg=seg16)


def _build(prep, num_devices=NCORES):
    import concourse.bacc as bacc
    import concourse.mybir as mybir
    import concourse.tile as tile
    from concourse import library_config

    dt = mybir.dt
    T = prep["T"]
    chunk_tiles = prep["chunk_tiles"]
    tile_w0 = prep["tile_w0"]
    TQMAX = max(te - ts for ts, te in chunk_tiles)

    nc = bacc.Bacc("TRN2", target_bir_lowering=False, debug=False,
                   num_devices=num_devices)
    X_d = nc.dram_tensor("x16", [N, C], dt.bfloat16, kind="ExternalInput")
    idx_d = nc.dram_tensor("idxw", [16, T * 8], dt.int16, kind="ExternalInput")
    seg_d = nc.dram_tensor("seg", [128, T * W], dt.bfloat16, kind="ExternalInput")
    wm_d = nc.dram_tensor("wm", [128, 3 * C], dt.bfloat16, kind="ExternalInput")
    out_d = nc.dram_tensor("out", [N, C], dt.float32, kind="ExternalOutput")

    with tile.TileContext(nc) as tc:
        with (
            tc.tile_pool(name="const", bufs=1) as constp,
            tc.tile_pool(name="xj", bufs=2) as xjp,
            tc.tile_pool(name="segp", bufs=2) as segp,
            tc.tile_pool(name="psg", bufs=2, space="PSUM") as psgp,
            tc.tile_pool(name="psx", bufs=2, space="PSUM") as psxp,
            tc.tile_pool(name="pso", bufs=2, space="PSUM") as psop,
            tc.tile_pool(name="dram", bufs=1, space="DRAM") as dramp,
        ):
            nc.gpsimd.load_library(library_config.mlp)

            wm = constp.tile([128, 3 * C], dt.bfloat16)
            nc.sync.dma_start(wm[:], wm_d[:])
            w1b = wm[:, 0:C]
            w1u = wm[:, C:2 * C]
            wv = wm[:, 2 * C:3 * C]

            idx_t = constp.tile([128, T * 8], dt.int16)
            for k in range(8):
                nc.sync.dma_start(idx_t[16 * k:16 * k + 16, :], idx_d[:])

            XbT = constp.tile([128, NP], dt.bfloat16)
            nc.sync.dma_start_transpose(XbT[:, :N], X_d[:])
            nc.vector.memset(XbT[:, N:], 0.0)

            xrows = constp.tile([128, NP], dt.bfloat16)
            G = constp.tile([128, NP], dt.bfloat16)
            nc.vector.memset(G[:, N:], 0.0)
            outr = constp.tile([128, NP], dt.bfloat16)
            sq = constp.tile([128, NP], dt.bfloat16)
            m12 = constp.tile([128, 160], dt.float32)
            m12s = constp.tile([128, 160], dt.float32)
            stats = constp.tile([128, 512], dt.float32)
            nc.vector.memset(m12[:], 0.0)

            # main pipeline: per 512-target chunk gather -> scatter -> out_row
            for q in range(NCHUNK):
                ts, te = chunk_tiles[q]
                nq = te - ts
                nt0 = 4 * q
                jn = min(4, NTILE - nt0)   # node tiles in this chunk

                xj = xjp.tile([128, TQMAX * 128], dt.bfloat16, tag="xj")
                xj_v = xj[:, :nq * 128].rearrange("p (t c) -> p t c", c=128)
                nc.gpsimd.dma_gather(
                    xj_v, X_d[:], idx_t[:, ts * 8:te * 8],
                    nq * 128, nq * 128, C,
                    single_packet=False,
                )
                sg = segp.tile([128, TQMAX * W], dt.bfloat16, tag="sg")
                nc.sync.dma_start(sg[:, :nq * W], seg_d[:, ts * W:te * W])

                chw = min(CHUNK_T, N - q * CHUNK_T)
                ps = psgp.tile([128, 512], dt.float32, tag="psg")
                nc.vector.memset(ps[:, :chw], 0.0)
                for t in range(nq):
                    w0 = tile_w0[ts + t]
                    nc.tensor.matmul(
                        ps[:, w0:w0 + W],
                        lhsT=xj[:, (t * 128):(t * 128 + 128)],
                        rhs=sg[:, t * W:(t + 1) * W],
                        start=False, stop=False, skip_group_check=True,
                    )
                nc.scalar.copy(G[:, q * CHUNK_T: q * CHUNK_T + chw],
                               ps[:, :chw])

                po = psop.tile([128, 512], dt.float32, tag="pso")
                for j in range(jn):
                    nt = nt0 + j
                    sl = slice(j * 128, (j + 1) * 128)
                    nc.tensor.matmul(po[:, sl], lhsT=XbT[:, nt * 128:(nt + 1) * 128],
                                     rhs=w1u, start=(j == 0), stop=False)
                    nc.tensor.matmul(po[:, sl], lhsT=G[:, nt * 128:(nt + 1) * 128],
                                     rhs=wv, start=False, stop=(j == jn - 1))
                nc.scalar.copy(outr[:, nt0 * 128: nt0 * 128 + jn * 128],
                               po[:, :jn * 128])

            # x rows (residual term), off critical path
            for g in range(NCHUNK):
                nt0 = 4 * g
                jn = min(4, NTILE - nt0)
                ps = psxp.tile([128, 512], dt.float32, tag="psx")
                for j in range(jn):
                    nt = nt0 + j
                    nc.tensor.matmul(ps[:, j * 128:(j + 1) * 128],
                                     lhsT=XbT[:, nt * 128:(nt + 1) * 128],
                                     rhs=w1b, start=(j == 0), stop=(j == jn - 1))
                nc.scalar.copy(xrows[:, nt0 * 128: nt0 * 128 + jn * 128],
                               ps[:, :jn * 128])

            # BN stats: m1 = sum_c out, m2 = sum_c out^2 per node
            outr_v = outr[:].rearrange("p (t c) -> p t c", c=128)
            nc.vector.tensor_reduce(m12[:, 0:NTILE], outr_v,
                                    axis=mybir.AxisListType.X,
                                    op=mybir.AluOpType.add)
            nc.vector.tensor_mul(sq[:], outr[:], outr[:])
            sq_v = sq[:].rearrange("p (t c) -> p t c", c=128)
            nc.vector.tensor_reduce(m12[:, 80:80 + NTILE], sq_v,
                                    axis=mybir.AxisListType.X,
                                    op=mybir.AluOpType.add)

            # cross-core AllReduce of the stats
            arin = dramp.tile([128, 160], dt.float32)
            arout = dramp.tile([128, 160], dt.float32)
            nc.gpsimd.dma_start(arin[:], m12[:])
            nc.gpsimd.collective_compute(
                "AllReduce", mybir.AluOpType.add,
                replica_groups=[list(range(num_devices))],
                ins=[arin.opt()], outs=[arout.opt()],
            )
            nc.gpsimd.dma_start(m12s[:], arout[:])

            mean = stats[:, 0:NTILE]
            ms = stats[:, 80:80 + NTILE]
            tmp = stats[:, 160:160 + NTILE]
            sd = stats[:, 240:240 + NTILE]
            istd = stats[:, 320:320 + NTILE]
            nb = stats[:, 400:400 + NTILE]
            zb = stats[:, 480:481]
            nc.vector.memset(zb, 0.0)
            nc.vector.tensor_scalar_mul(mean, m12s[:, 0:NTILE], 1.0 / DENOM)
            nc.vector.tensor_scalar_mul(ms, m12s[:, 80:80 + NTILE], 1.0 / DENOM)
            nc.vector.tensor_mul(tmp, mean, mean)
            nc.vector.tensor_sub(ms, ms, tmp)
            nc.vector.tensor_scalar_add(ms, ms, EPS)
            nc.scalar.activation(sd, ms, mybir.ActivationFunctionType.Sqrt,
                                 bias=zb)
            nc.vector.reciprocal(istd, sd)
            nc.vector.tensor_mul(nb, mean, istd)
            nc.vector.tensor_scalar_mul(nb, nb, -1.0)

            # normalize + relu per node tile (per-partition scale/bias)
            for nt in range(NTILE):
                sl = slice(nt * 128, (nt + 1) * 128)
                nc.scalar.activation(outr[:, sl], outr[:, sl],
                                     mybir.ActivationFunctionType.Relu,
                                     bias=nb[:, nt:nt + 1],
                                     scale=istd[:, nt:nt + 1])
            nc.vector.tensor_add(outr[:], outr[:], xrows[:])

            # output: [p, t, c] -> HBM row-major [N, C] fp32 (SWDGE cast)
            nfull = (N // 128) * 128  # 9984
            tfull = N // 128          # 78
            out_main = out_d[:nfull, :].rearrange("(t p) c -> p t c", p=128)
            src_main = outr[:, :tfull * 128].rearrange("p (t c) -> p t c", c=128)
            nc.gpsimd.dma_start(out_main, src_main)
            rem = N - nfull           # 16
            if rem:
                nc.gpsimd.dma_start(out_d[nfull:N, :],
                                    outr[0:rem, tfull * 128:(tfull + 1) * 128])

    nc.compile()
    return nc


def _get_state(edge_index, edge_weight):
    global _state
    key = hashlib.sha1(np.ascontiguousarray(edge_index).tobytes()).hexdigest()
    if _state is None or _state["key"] != key:
        prep = _preprocess(edge_index, edge_weight)
        nc = _build(prep)
        _state = {"key": key, "prep": prep, "nc": nc}
    return _state


def make_in_maps(X, edge_index, edge_weight, weight1, weight2, u, v, prep):
    w1 = np.asarray(weight1, np.float32)
    u_ = np.asarray(u, np.float32)
    v_ = np.asarray(v, np.float32)
    w2 = np.asarray(weight2, np.float32)
    wm = np.concatenate(
        [w1.astype(bf16),
         (w1 @ u_).astype(bf16),
         (w1 @ v_ * w2[0][None, :]).astype(bf16)], axis=1)
    wm = np.ascontiguousarray(wm)
    X16 = np.asarray(X, np.float32).astype(bf16)
    return [
        {"x16": np.ascontiguousarray(X16[b]), "idxw": prep["idxw"],
         "seg": prep["seg"], "wm": wm}
        for b in range(B)
    ]


def kernel(X, edge_index, edge_weight, weight1, weight2, u, v):
    from concourse import bass_utils

    st = _get_state(edge_index, edge_weight)
    in_maps = make_in_maps(X, edge_index, edge_weight, weight1, weight2, u, v,
                           st["prep"])
    res = bass_utils.run_bass_kernel_spmd(
        st["nc"], in_maps, core_ids=list(range(NCORES)))
    return np.stack([res.results[b]["out"] for b in range(B)]).astype(np.float32)
